# revision 12
# baseline (speedup 1.0000x reference)
"""CRF decoder loss kernel for Trainium2 (8 NeuronCores, data-parallel over batch).

Algorithm (mathematically identical to the reference):
  The reference computes mean_b(Zp - score) where Zp is the CRF partition
  function of log_softmax(enc@W+b) and score is the gold-path score. Writing
  logits = R - logZ (R the raw projection scores, logZ the log-softmax
  normalizer), the normalizer cancels between Zp and score, so no softmax is
  ever needed. With a constant shift kappa for range control, the forward
  recursion runs in LINEAR space:

      P_0 = exp(start) * G_0,     P_t = (P_{t-1} @ exp(T)) * G_t,
      G_t = exp(R_t - kappa)                                  (all [B, V])

  loss_b = log(S[len_b-1, b]) with S[t] = P_t @ exp(end)      <- device
           - sum_{t<len_b} (R[t,b,tgt_{t,b}] - kappa)         <- host (tiny)
           - (start[tgt_0] + sum T[tgt,tgt'] + end[tgt_last]) <- host (tiny)

Device schedule per core (batch shard of 32):
  Phase 1 (projection): all 32 chunks back-to-back so the PE ramps to its
  full p-state; R^T = W^T @ encT into PSUM (bf16, fp32 accum), ACT evicts
  G^T = exp(R^T + (b - kappa)) as bf16 into a per-step-contiguous layout.
  Phase 2 (scan): two interleaved 16-batch groups so each engine has work
  adjacent to its semaphore-gated ops (hiding the PE 173ns / DVE 125ns
  pipeline-restart latencies); per group-step 4 matmuls with the 128x128
  blocks of exp(T) and ONE flat [128,32] DVE multiply by G_t^T. S_t is
  extracted every step by 2 tiny matmuls with exp(end) + an ACT copy.
"""

import numpy as np
import ml_dtypes

import concourse.bacc as bacc
import concourse.tile as tile
from concourse import mybir
from concourse.bass_utils import run_bass_kernel_spmd

bf16 = ml_dtypes.bfloat16
f32 = mybir.dt.float32
bf16_t = mybir.dt.bfloat16

S, B, H, V = 512, 256, 512, 256
NCORES = 8
BC = B // NCORES            # 32 batch per core
ROWS = S * BC               # 16384 rows (t-major, b-minor)
KAPPA = 6.05
CHUNK = 512                 # projection chunk (rows)
NCHUNK = ROWS // CHUNK      # 32
SBLK = 16                   # scan steps per projection chunk
RING = 4                    # state ring slots
NG = 2                      # interleaved scan batch groups (pipeline stages)
GB = BC // NG               # 16 batch per group

_nc_cache = None


def _build():
    nc = bacc.Bacc("TRN2", debug=False)

    encT = nc.dram_tensor("encT", [128, NCHUNK, 4, CHUNK], bf16_t, kind="ExternalInput")
    wblk = nc.dram_tensor("wblk", [128, 8, 128], bf16_t, kind="ExternalInput")
    expTblk = nc.dram_tensor("expTblk", [128, 4, 128], bf16_t, kind="ExternalInput")
    biasT = nc.dram_tensor("biasT", [128, 2], f32, kind="ExternalInput")
    expStartT = nc.dram_tensor("expStartT", [128, 2], f32, kind="ExternalInput")
    expEndT = nc.dram_tensor("expEndT", [128, 2], bf16_t, kind="ExternalInput")

    s_out = nc.dram_tensor("s_out", [1, ROWS], f32, kind="ExternalOutput")

    with tile.TileContext(nc) as tc:
        with (
            tc.tile_pool(name="consts", bufs=1) as consts,
            tc.tile_pool(name="encp", bufs=4) as encp,
            tc.tile_pool(name="gpool", bufs=1) as gpool,
            tc.tile_pool(name="scan_ps", bufs=2, space="PSUM") as scan_ps,
            tc.tile_pool(name="s_ps", bufs=2, space="PSUM") as s_ps,
        ):
            w_sb = consts.tile([128, 8, 128], bf16_t)
            expT_sb = consts.tile([128, 4, 128], bf16_t)
            bias_sb = consts.tile([128, 2], f32)
            expStart_sb = consts.tile([128, 2], f32)
            expEnd_sb = consts.tile([128, 2], bf16_t)
            s_sb = consts.tile([1, ROWS], f32)
            # ring free layout: (slot, group, jh, b') -> per (slot, group)
            # the 2*GB state columns are CONTIGUOUS (flat DVE access)
            ring = consts.tile([128, RING, NG, 2, GB], bf16_t)

            nc.sync.dma_start(out=w_sb[:], in_=wblk[:])
            nc.sync.dma_start(out=expT_sb[:], in_=expTblk[:])
            nc.sync.dma_start(out=bias_sb[:], in_=biasT[:])
            nc.sync.dma_start(out=expStart_sb[:], in_=expStartT[:])
            nc.sync.dma_start(out=expEnd_sb[:], in_=expEndT[:])

            # ---------------- phase 1: projection (all chunks) ----------------
            # G chunk free layout: (t%16, group, jh, b') — per (t, group)
            # contiguous 32 columns, matching the ring layout.
            gtiles = []
            with tc.tile_pool(name="proj_ps", bufs=4, space="PSUM") as proj_ps:
                for c in range(NCHUNK):
                    et = encp.tile([128, 4, CHUNK], bf16_t, name="et", tag="enc")
                    nc.sync.dma_start(out=et[:], in_=encT[:, c, :, :])
                    g = gpool.tile([128, SBLK, NG, 2, GB], bf16_t,
                                   name=f"g{c}", tag=f"g{c}")
                    gtiles.append(g)
                    for vh in range(2):
                        ps = proj_ps.tile([128, CHUNK], f32, name="pps", tag="pps")
                        for ht in range(4):
                            nc.tensor.matmul(
                                ps[:],
                                lhsT=w_sb[:, ht * 2 + vh, :],
                                rhs=et[:, ht, :],
                                start=(ht == 0),
                                stop=(ht == 3),
                            )
                        # psum cols r = t*32 + g*16 + b' stream in order into
                        # out free dims (t, g, b') at strides (64, 32, 1)
                        nc.scalar.activation(
                            g[:, :, :, vh, :], ps[:],
                            mybir.ActivationFunctionType.Exp,
                            bias=bias_sb[:, vh:vh + 1], scale=1.0,
                        )

            # ---------------- phase 2: scan ----------------
            for gi in range(NG):
                for ih in range(2):
                    nc.vector.tensor_scalar_mul(
                        ring[:, 0, gi, ih, :],
                        in0=gtiles[0][:, 0, gi, ih, :],
                        scalar1=expStart_sb[:, ih:ih + 1],
                    )

            def emit_s(t):
                # S_t[b] = sum_v P_t^T[v, b] * expEnd[v]  (all 32 b at once)
                sp = s_ps.tile([1, BC], f32, name="sps", tag="sps")
                for ih in range(2):
                    nc.tensor.matmul(
                        sp[:],
                        lhsT=expEnd_sb[:, ih:ih + 1],
                        rhs=ring[:, t % RING, :, ih, :],
                        start=(ih == 0),
                        stop=(ih == 1),
                    )
                nc.scalar.copy(s_sb[0:1, t * BC:(t + 1) * BC], sp[:])

            emit_s(0)
            for t in range(1, S):
                gt = gtiles[t // SBLK]
                ti = t % SBLK
                for gi in range(NG):
                    ps = scan_ps.tile([128, 2, GB], f32, name=f"ps{gi}",
                                      tag="ps")
                    for jh in range(2):
                        for ih in range(2):
                            nc.tensor.matmul(
                                ps[:, jh, :],
                                lhsT=expT_sb[:, ih * 2 + jh, :],
                                rhs=ring[:, (t - 1) % RING, gi, ih, :],
                                start=(ih == 0),
                                stop=(ih == 1),
                            )
                    # flat [128, 32] multiply: ps (2,16) and ring/G slices are
                    # contiguous 32-column runs
                    nc.vector.tensor_tensor(
                        out=ring[:, t % RING, gi, :, :],
                        in0=ps[:],
                        in1=gt[:, ti, gi, :, :],
                        op=mybir.AluOpType.mult,
                    )
                emit_s(t)

            nc.sync.dma_start(out=s_out[:], in_=s_sb[:])

    nc.compile()
    return nc


def _host_consts(d):
    W_ = np.asarray(d["W"], dtype=np.float32)
    b_ = np.asarray(d["b"], dtype=np.float64)
    T_ = np.asarray(d["transition"], dtype=np.float64)
    start_ = np.asarray(d["start_transition"], dtype=np.float64)
    end_ = np.asarray(d["end_transition"], dtype=np.float64)
    Wb = np.ascontiguousarray(
        W_.reshape(4, 128, 2, 128).transpose(1, 0, 2, 3).reshape(128, 8, 128)
    ).astype(bf16)
    expTb = np.ascontiguousarray(
        np.exp(T_).reshape(2, 128, 2, 128).transpose(1, 0, 2, 3).reshape(128, 4, 128)
    ).astype(bf16)
    biasT = np.ascontiguousarray(
        (b_ - KAPPA).reshape(2, 128).T).astype(np.float32)
    expStartT = np.ascontiguousarray(
        np.exp(start_).reshape(2, 128).T).astype(np.float32)
    expEndT = np.ascontiguousarray(
        np.exp(end_).reshape(2, 128).T).astype(bf16)
    return Wb, expTb, biasT, expStartT, expEndT


def _prep_core_inputs(core, enc_bf, Wb, expTb, biasT, expStartT, expEndT):
    # encT layout [h%128, chunk, h//128, row-in-chunk]; rows are t*BC + b
    b0 = core * BC
    e = enc_bf[:, b0:b0 + BC, :].transpose(2, 0, 1).reshape(4, 128, NCHUNK, CHUNK)
    e = np.ascontiguousarray(e.transpose(1, 2, 0, 3))
    return {
        "encT": e, "wblk": Wb, "expTblk": expTb, "biasT": biasT,
        "expStartT": expStartT, "expEndT": expEndT,
    }


def kernel(enc_outs, W, b, transition, start_transition, end_transition,
           targets, lengths):
    global _nc_cache
    if _nc_cache is None:
        _nc_cache = _build()
    nc = _nc_cache

    enc = np.asarray(enc_outs, dtype=np.float32)
    W_ = np.asarray(W, dtype=np.float32)
    b_ = np.asarray(b, dtype=np.float64)
    T_ = np.asarray(transition, dtype=np.float64)
    start_ = np.asarray(start_transition, dtype=np.float64)
    end_ = np.asarray(end_transition, dtype=np.float64)
    tgt = np.asarray(targets).astype(np.int64)
    lens = np.asarray(lengths).astype(np.int64)

    Wb, expTb, biasT, expStartT, expEndT = _host_consts({
        "W": W, "b": b, "transition": transition,
        "start_transition": start_transition, "end_transition": end_transition,
    })
    enc_bf = enc.astype(bf16)
    in_maps = [
        _prep_core_inputs(c, enc_bf, Wb, expTb, biasT, expStartT, expEndT)
        for c in range(NCORES)
    ]
    res = run_bass_kernel_spmd(nc, in_maps, list(range(NCORES))).results

    # ---------------- host epilogue (small inputs only) ----------------
    tmask = (np.arange(S)[:, None] < lens[None, :])
    trans_sum = (T_[tgt[:-1], tgt[1:]] * tmask[1:]).sum(axis=0)
    last_tgt = tgt[lens - 1, np.arange(B)]
    hostscore = start_[tgt[0]] + trans_sum + end_[last_tgt]

    # gold-path raw emission scores: R[t, b, tgt] = enc[t, b] . W[:, tgt] + b
    # (16K dot products per core; 0.1% of the device FLOPs)
    Wg = W_.T[tgt.reshape(-1)]                        # (S*B, H)
    emis_all = (np.einsum("rh,rh->r", enc.reshape(S * B, H), Wg,
                          optimize=True).reshape(S, B)
                + b_[tgt])
    emis = ((emis_all - KAPPA) * tmask).sum(axis=0)

    loss_b = np.zeros(B, dtype=np.float64)
    for c in range(NCORES):
        b0 = c * BC
        s_flat = np.asarray(res[c]["s_out"], dtype=np.float64).reshape(S, BC)
        bl = lens[b0:b0 + BC] - 1
        s_end = s_flat[bl, np.arange(BC)]
        loss_b[b0:b0 + BC] = np.log(s_end) - emis[b0:b0 + BC] \
            - hostscore[b0:b0 + BC]

    return np.float32(loss_b.mean())


# revision 13
# speedup vs baseline: 1.1318x; 1.1318x over previous
"""CRF decoder loss kernel for Trainium2 (8 NeuronCores, data-parallel over batch).

Algorithm (mathematically identical to the reference):
  The reference computes mean_b(Zp - score) where Zp is the CRF partition
  function of log_softmax(enc@W+b) and score is the gold-path score. Writing
  logits = R - logZ (R the raw projection scores, logZ the log-softmax
  normalizer), the normalizer cancels between Zp and score, so no softmax is
  ever needed. With a constant shift kappa for range control, the forward
  recursion runs in LINEAR space:

      P_0 = exp(start) * G_0,     P_t = (P_{t-1} @ exp(T)) * G_t,
      G_t = exp(R_t - kappa)                                  (all [B, V])

  loss_b = log(S[len_b-1, b]) with S[t] = P_t @ exp(end)      <- device
           - sum_{t<len_b} (R[t,b,tgt_{t,b}] - kappa)         <- host (tiny)
           - (start[tgt_0] + sum T[tgt,tgt'] + end[tgt_last]) <- host (tiny)

Device schedule per core (batch shard of 32):
  - projection: R^T = W^T @ encT (bf16, fp32 accum), ACT evicts
    G^T = exp(R^T + (b - kappa)) as bf16. Chunks 0-6 run dense up front;
    the rest is emitted as free-256 matmul slices between scan steps.
  - scan: two interleaved 16-batch groups; per group-step 4 matmuls with the
    128x128 blocks of exp(T) and ONE flat [128,32] DVE multiply by G_t^T.
    All 512 states are kept (64KB/partition) — no ring reuse.
  - The scan's serial cycle is sem -> PE matmuls -> sem -> DVE multiply;
    the PE's 173ns and DVE's 125ns pipeline-restart latencies are hidden by
    keeping each engine busy across its semaphore-gated issue: group B's
    matmuls ride behind group A's, and ~2 dependency-free filler matmuls
    (projection slices / S-extraction over old states) pad the PE queue
    between step t's and step t+1's scan matmuls.
  - S extraction: per 16-step block, 8 matmuls of exp(end) against old
    state slots (free 128) + an ACT copy into the staging row.
"""

import numpy as np
import ml_dtypes

import concourse.bacc as bacc
import concourse.tile as tile
from concourse import mybir
from concourse.bass_utils import run_bass_kernel_spmd

bf16 = ml_dtypes.bfloat16
f32 = mybir.dt.float32
bf16_t = mybir.dt.bfloat16

S, B, H, V = 512, 256, 512, 256
NCORES = 8
BC = B // NCORES            # 32 batch per core
ROWS = S * BC               # 16384 rows (t-major, b-minor)
KAPPA = 6.05
CHUNK = 512                 # projection chunk (rows)
NCHUNK = ROWS // CHUNK      # 32
SBLK = 16                   # scan steps per projection chunk
NG = 2                      # interleaved scan batch groups (pipeline stages)
GB = BC // NG               # 16 batch per group
LEAD = 7                    # chunks projected densely before the scan

_nc_cache = None


def _build():
    nc = bacc.Bacc("TRN2", debug=False)

    encT = nc.dram_tensor("encT", [128, NCHUNK, 4, CHUNK], bf16_t, kind="ExternalInput")
    wblk = nc.dram_tensor("wblk", [128, 8, 128], bf16_t, kind="ExternalInput")
    expTblk = nc.dram_tensor("expTblk", [128, 4, 128], bf16_t, kind="ExternalInput")
    biasT = nc.dram_tensor("biasT", [128, 2], f32, kind="ExternalInput")
    expStartT = nc.dram_tensor("expStartT", [128, 2], f32, kind="ExternalInput")
    expEndT = nc.dram_tensor("expEndT", [128, 2], bf16_t, kind="ExternalInput")

    s_out = nc.dram_tensor("s_out", [1, ROWS], f32, kind="ExternalOutput")

    with tile.TileContext(nc) as tc:
        with (
            tc.tile_pool(name="consts", bufs=1) as consts,
            tc.tile_pool(name="encp", bufs=4) as encp,
            tc.tile_pool(name="gpool", bufs=LEAD) as gpool,
            tc.tile_pool(name="scan_ps", bufs=2, space="PSUM") as scan_ps,
            tc.tile_pool(name="proj_ps", bufs=2, space="PSUM") as proj_ps,
            tc.tile_pool(name="s_ps", bufs=2, space="PSUM") as s_ps,
        ):
            w_sb = consts.tile([128, 8, 128], bf16_t)
            expT_sb = consts.tile([128, 4, 128], bf16_t)
            bias_sb = consts.tile([128, 2], f32)
            expStart_sb = consts.tile([128, 2], f32)
            expEnd_sb = consts.tile([128, 2], bf16_t)
            s_sb = consts.tile([1, ROWS], f32)
            # all scan states, free layout (slot=t, group, jh, b') -> per
            # (slot, group) the 2*GB state columns are CONTIGUOUS
            states = consts.tile([128, S, NG, 2, GB], bf16_t)

            nc.sync.dma_start(out=w_sb[:], in_=wblk[:])
            nc.sync.dma_start(out=expT_sb[:], in_=expTblk[:])
            nc.sync.dma_start(out=bias_sb[:], in_=biasT[:])
            nc.sync.dma_start(out=expStart_sb[:], in_=expStartT[:])
            nc.sync.dma_start(out=expEnd_sb[:], in_=expEndT[:])

            gtiles = []
            fillers = []

            def push_proj_chunk(c):
                """DMA now; 16 free-256 matmul slices + 2 exps as fillers."""
                et = encp.tile([128, 4, CHUNK], bf16_t, name="et", tag="enc")
                nc.sync.dma_start(out=et[:], in_=encT[:, c, :, :])
                # G chunk free layout (t%16, group, jh, b'): per (t, group)
                # contiguous 32 columns, matching the states layout
                g = gpool.tile([128, SBLK, NG, 2, GB], bf16_t,
                               name=f"g{c}", tag="g")
                gtiles.append(g)
                ps_box = [None, None]

                def mk_mm(vh, sl, ht):
                    def mm():
                        if sl == 0 and ht == 0:
                            ps_box[vh] = proj_ps.tile(
                                [128, CHUNK], f32, name="pps", tag="pps")
                        nc.tensor.matmul(
                            ps_box[vh][:, sl * 256:(sl + 1) * 256],
                            lhsT=w_sb[:, ht * 2 + vh, :],
                            rhs=et[:, ht, sl * 256:(sl + 1) * 256],
                            start=(ht == 0),
                            stop=(ht == 3),
                        )
                    return mm

                def mk_exp(vh):
                    def ex():
                        # psum cols r = t*32 + g*16 + b' stream in order into
                        # out free dims (t, g, b') at strides (64, 32, 1)
                        nc.scalar.activation(
                            g[:, :, :, vh, :], ps_box[vh][:],
                            mybir.ActivationFunctionType.Exp,
                            bias=bias_sb[:, vh:vh + 1], scale=1.0,
                        )
                    return ex

                for vh in range(2):
                    for sl in range(2):
                        for ht in range(4):
                            fillers.append(mk_mm(vh, sl, ht))
                    fillers.append(mk_exp(vh))

            def push_sblock(k):
                """S_t for t in block k: 8 free-128 matmuls + 1 copy."""
                sp = s_ps.tile([1, SBLK * BC], f32, name="sps", tag="sps")
                s0 = k * SBLK

                def mk_mm(half, gi, ih):
                    def mm():
                        nc.tensor.matmul(
                            sp[0:1, (half * 2 + gi) * 128:
                               (half * 2 + gi + 1) * 128],
                            lhsT=expEnd_sb[:, ih:ih + 1],
                            rhs=states[:, s0 + half * 8:s0 + (half + 1) * 8,
                                       gi, ih, :],
                            start=(ih == 0),
                            stop=(ih == 1),
                        )
                    return mm

                def cp():
                    nc.scalar.copy(
                        s_sb[0:1, k * (SBLK * BC):(k + 1) * (SBLK * BC)],
                        sp[:])

                for half in range(2):
                    for gi in range(NG):
                        for ih in range(2):
                            fillers.append(mk_mm(half, gi, ih))
                fillers.append(cp)

            def filler_tick(n=2):
                for _ in range(n):
                    if fillers:
                        fillers.pop(0)()

            # ------------- prologue: chunks 0..LEAD-1 densely -------------
            for c in range(LEAD):
                push_proj_chunk(c)
            while fillers:
                filler_tick()

            # ---------------- scan ----------------
            for gi in range(NG):
                for ih in range(2):
                    nc.vector.tensor_scalar_mul(
                        states[:, 0, gi, ih, :],
                        in0=gtiles[0][:, 0, gi, ih, :],
                        scalar1=expStart_sb[:, ih:ih + 1],
                    )

            for t in range(1, S):
                if t % SBLK == 1:
                    blk = t // SBLK
                    if blk + LEAD < NCHUNK:
                        push_proj_chunk(blk + LEAD)
                    if blk >= 1:
                        push_sblock(blk - 1)

                gt = gtiles[t // SBLK]
                ti = t % SBLK
                for gi in range(NG):
                    ps = scan_ps.tile([128, 2, GB], f32, name=f"ps{gi}",
                                      tag="ps")
                    for jh in range(2):
                        for ih in range(2):
                            nc.tensor.matmul(
                                ps[:, jh, :],
                                lhsT=expT_sb[:, ih * 2 + jh, :],
                                rhs=states[:, t - 1, gi, ih, :],
                                start=(ih == 0),
                                stop=(ih == 1),
                            )
                    # flat [128, 32] multiply: ps and the states/G slices
                    # are contiguous 32-column runs
                    nc.vector.tensor_tensor(
                        out=states[:, t, gi, :, :],
                        in0=ps[:],
                        in1=gt[:, ti, gi, :, :],
                        op=mybir.AluOpType.mult,
                    )
                filler_tick()

            push_sblock(NCHUNK - 1)
            while fillers:
                filler_tick()

            nc.sync.dma_start(out=s_out[:], in_=s_sb[:])

    nc.compile()
    return nc


def _host_consts(d):
    W_ = np.asarray(d["W"], dtype=np.float32)
    b_ = np.asarray(d["b"], dtype=np.float64)
    T_ = np.asarray(d["transition"], dtype=np.float64)
    start_ = np.asarray(d["start_transition"], dtype=np.float64)
    end_ = np.asarray(d["end_transition"], dtype=np.float64)
    Wb = np.ascontiguousarray(
        W_.reshape(4, 128, 2, 128).transpose(1, 0, 2, 3).reshape(128, 8, 128)
    ).astype(bf16)
    expTb = np.ascontiguousarray(
        np.exp(T_).reshape(2, 128, 2, 128).transpose(1, 0, 2, 3).reshape(128, 4, 128)
    ).astype(bf16)
    biasT = np.ascontiguousarray(
        (b_ - KAPPA).reshape(2, 128).T).astype(np.float32)
    expStartT = np.ascontiguousarray(
        np.exp(start_).reshape(2, 128).T).astype(np.float32)
    expEndT = np.ascontiguousarray(
        np.exp(end_).reshape(2, 128).T).astype(bf16)
    return Wb, expTb, biasT, expStartT, expEndT


def _prep_core_inputs(core, enc_bf, Wb, expTb, biasT, expStartT, expEndT):
    # encT layout [h%128, chunk, h//128, row-in-chunk]; rows are t*BC + b
    b0 = core * BC
    e = enc_bf[:, b0:b0 + BC, :].transpose(2, 0, 1).reshape(4, 128, NCHUNK, CHUNK)
    e = np.ascontiguousarray(e.transpose(1, 2, 0, 3))
    return {
        "encT": e, "wblk": Wb, "expTblk": expTb, "biasT": biasT,
        "expStartT": expStartT, "expEndT": expEndT,
    }


def kernel(enc_outs, W, b, transition, start_transition, end_transition,
           targets, lengths):
    global _nc_cache
    if _nc_cache is None:
        _nc_cache = _build()
    nc = _nc_cache

    enc = np.asarray(enc_outs, dtype=np.float32)
    W_ = np.asarray(W, dtype=np.float32)
    b_ = np.asarray(b, dtype=np.float64)
    T_ = np.asarray(transition, dtype=np.float64)
    start_ = np.asarray(start_transition, dtype=np.float64)
    end_ = np.asarray(end_transition, dtype=np.float64)
    tgt = np.asarray(targets).astype(np.int64)
    lens = np.asarray(lengths).astype(np.int64)

    Wb, expTb, biasT, expStartT, expEndT = _host_consts({
        "W": W, "b": b, "transition": transition,
        "start_transition": start_transition, "end_transition": end_transition,
    })
    enc_bf = enc.astype(bf16)
    in_maps = [
        _prep_core_inputs(c, enc_bf, Wb, expTb, biasT, expStartT, expEndT)
        for c in range(NCORES)
    ]
    res = run_bass_kernel_spmd(nc, in_maps, list(range(NCORES))).results

    # ---------------- host epilogue (small inputs only) ----------------
    tmask = (np.arange(S)[:, None] < lens[None, :])
    trans_sum = (T_[tgt[:-1], tgt[1:]] * tmask[1:]).sum(axis=0)
    last_tgt = tgt[lens - 1, np.arange(B)]
    hostscore = start_[tgt[0]] + trans_sum + end_[last_tgt]

    # gold-path raw emission scores: R[t, b, tgt] = enc[t, b] . W[:, tgt] + b
    # (16K dot products per core; 0.1% of the device FLOPs)
    Wg = W_.T[tgt.reshape(-1)]                        # (S*B, H)
    emis_all = (np.einsum("rh,rh->r", enc.reshape(S * B, H), Wg,
                          optimize=True).reshape(S, B)
                + b_[tgt])
    emis = ((emis_all - KAPPA) * tmask).sum(axis=0)

    loss_b = np.zeros(B, dtype=np.float64)
    for c in range(NCORES):
        b0 = c * BC
        # S col layout: (blk, half, g, ti, b') with t = blk*16+half*8+ti,
        # b = g*16 + b'
        s_dec = np.asarray(res[c]["s_out"], dtype=np.float64).reshape(
            S // SBLK, 2, NG, 8, GB)
        bl = lens[b0:b0 + BC] - 1
        blocal = np.arange(BC)
        s_end = s_dec[bl // SBLK, (bl % SBLK) // 8, blocal // GB,
                      bl % 8, blocal % GB]
        loss_b[b0:b0 + BC] = np.log(s_end) - emis[b0:b0 + BC] \
            - hostscore[b0:b0 + BC]

    return np.float32(loss_b.mean())


# revision 18
# speedup vs baseline: 1.1507x; 1.0167x over previous
"""CRF decoder loss kernel for Trainium2 (8 NeuronCores, data-parallel over batch).

Algorithm (mathematically identical to the reference):
  The reference computes mean_b(Zp - score) where Zp is the CRF partition
  function of log_softmax(enc@W+b) and score is the gold-path score. Writing
  logits = R - logZ (R the raw projection scores, logZ the log-softmax
  normalizer), the normalizer cancels between Zp and score, so no softmax is
  ever needed. With a constant shift kappa for range control, the forward
  recursion runs in LINEAR space:

      P_0 = exp(start) * G_0,     P_t = (P_{t-1} @ exp(T)) * G_t,
      G_t = exp(R_t - kappa)                                  (all [B, V])

  loss_b = log(S[len_b-1, b]) with S[t] = P_t @ exp(end)      <- device
           - sum_{t<len_b} (R[t,b,tgt_{t,b}] - kappa)         <- host (tiny)
           - (start[tgt_0] + sum T[tgt,tgt'] + end[tgt_last]) <- host (tiny)

Device schedule per core (batch shard of 32):
  - projection: R^T = W^T @ encT (bf16, fp32 accum), ACT evicts
    G^T = exp(R^T + (b - kappa)) as bf16. Chunks 0-6 run dense up front;
    the rest is emitted as free-256 matmul slices between scan steps.
  - scan: two interleaved 16-batch groups; per group-step 4 matmuls with the
    128x128 blocks of exp(T) and ONE flat [128,32] DVE multiply by G_t^T.
    All 512 states are kept (64KB/partition) — no ring reuse.
  - The scan's serial cycle is sem -> PE matmuls -> sem -> DVE multiply;
    the PE's 173ns and DVE's 125ns pipeline-restart latencies are hidden by
    keeping each engine busy across its semaphore-gated issue: group B's
    matmuls ride behind group A's, and ~2 dependency-free filler matmuls
    (projection slices / S-extraction over old states) pad the PE queue
    between step t's and step t+1's scan matmuls.
  - S extraction: per 16-step block, 8 matmuls of exp(end) against old
    state slots (free 128) + an ACT copy into the staging row.
"""

import numpy as np
import ml_dtypes

import concourse.bacc as bacc
import concourse.tile as tile
from concourse import mybir
from concourse.bass_utils import run_bass_kernel_spmd

bf16 = ml_dtypes.bfloat16
f32 = mybir.dt.float32
bf16_t = mybir.dt.bfloat16

S, B, H, V = 512, 256, 512, 256
NCORES = 8
BC = B // NCORES            # 32 batch per core
ROWS = S * BC               # 16384 rows (t-major, b-minor)
KAPPA = 6.05
CHUNK = 512                 # projection chunk (rows)
NCHUNK = ROWS // CHUNK      # 32
SBLK = 16                   # scan steps per projection chunk
NG = 2                      # interleaved scan batch groups (pipeline stages)
GB = BC // NG               # 16 batch per group
LEAD = 4                    # chunks projected densely before the scan

_nc_cache = None


def _build():
    nc = bacc.Bacc("TRN2", debug=False)

    encT = nc.dram_tensor("encT", [128, NCHUNK, 4, CHUNK], bf16_t, kind="ExternalInput")
    wblk = nc.dram_tensor("wblk", [128, 8, 128], bf16_t, kind="ExternalInput")
    expTblk = nc.dram_tensor("expTblk", [128, 4, 128], bf16_t, kind="ExternalInput")
    biasT = nc.dram_tensor("biasT", [128, 2], f32, kind="ExternalInput")
    expStartT = nc.dram_tensor("expStartT", [128, 2], f32, kind="ExternalInput")
    expEndT = nc.dram_tensor("expEndT", [128, 2], bf16_t, kind="ExternalInput")

    s_out = nc.dram_tensor("s_out", [1, ROWS], f32, kind="ExternalOutput")

    with tile.TileContext(nc) as tc:
        with (
            tc.tile_pool(name="consts", bufs=1) as consts,
            tc.tile_pool(name="encp", bufs=4) as encp,
            tc.tile_pool(name="gpool", bufs=LEAD) as gpool,
            tc.tile_pool(name="scan_ps", bufs=2, space="PSUM") as scan_ps,
            tc.tile_pool(name="proj_ps", bufs=2, space="PSUM") as proj_ps,
            tc.tile_pool(name="s_ps", bufs=2, space="PSUM") as s_ps,
        ):
            w_sb = consts.tile([128, 8, 128], bf16_t)
            expT_sb = consts.tile([128, 4, 128], bf16_t)
            bias_sb = consts.tile([128, 2], f32)
            expStart_sb = consts.tile([128, 2], f32)
            expEnd_sb = consts.tile([128, 2], bf16_t)
            s_sb = consts.tile([1, ROWS], f32)
            # all scan states, free layout (slot=t, group, jh, b') -> per
            # (slot, group) the 2*GB state columns are CONTIGUOUS
            states = consts.tile([128, S, NG, 2, GB], bf16_t)

            nc.sync.dma_start(out=w_sb[:], in_=wblk[:])
            nc.sync.dma_start(out=expT_sb[:], in_=expTblk[:])
            nc.sync.dma_start(out=bias_sb[:], in_=biasT[:])
            nc.sync.dma_start(out=expStart_sb[:], in_=expStartT[:])
            nc.sync.dma_start(out=expEnd_sb[:], in_=expEndT[:])

            gtiles = []
            fillers = []

            def push_proj_chunk(c):
                """DMA now; 16 free-256 matmul slices + 2 exps as fillers."""
                et = encp.tile([128, 4, CHUNK], bf16_t, name="et", tag="enc")
                nc.sync.dma_start(out=et[:], in_=encT[:, c, :, :])
                # G chunk free layout (t%16, group, jh, b'): per (t, group)
                # contiguous 32 columns, matching the states layout
                g = gpool.tile([128, SBLK, NG, 2, GB], bf16_t,
                               name=f"g{c}", tag="g")
                gtiles.append(g)
                ps_box = [None, None]

                def mk_mm(vh, sl, ht):
                    def mm():
                        if sl == 0 and ht == 0:
                            ps_box[vh] = proj_ps.tile(
                                [128, CHUNK], f32, name="pps", tag="pps")
                        nc.tensor.matmul(
                            ps_box[vh][:, sl * 256:(sl + 1) * 256],
                            lhsT=w_sb[:, ht * 2 + vh, :],
                            rhs=et[:, ht, sl * 256:(sl + 1) * 256],
                            start=(ht == 0),
                            stop=(ht == 3),
                        )
                    return mm

                def mk_exp(vh):
                    def ex():
                        # psum cols r = t*32 + g*16 + b' stream in order into
                        # out free dims (t, g, b') at strides (64, 32, 1)
                        nc.scalar.activation(
                            g[:, :, :, vh, :], ps_box[vh][:],
                            mybir.ActivationFunctionType.Exp,
                            bias=bias_sb[:, vh:vh + 1], scale=1.0,
                        )
                    return ex

                for vh in range(2):
                    for sl in range(2):
                        for ht in range(4):
                            fillers.append(mk_mm(vh, sl, ht))
                    fillers.append(mk_exp(vh))

            def push_sblock(k):
                """S_t for t in block k: 8 free-128 matmuls + 1 copy."""
                sp = s_ps.tile([1, SBLK * BC], f32, name="sps", tag="sps")
                s0 = k * SBLK

                def mk_mm(half, gi, ih):
                    def mm():
                        nc.tensor.matmul(
                            sp[0:1, (half * 2 + gi) * 128:
                               (half * 2 + gi + 1) * 128],
                            lhsT=expEnd_sb[:, ih:ih + 1],
                            rhs=states[:, s0 + half * 8:s0 + (half + 1) * 8,
                                       gi, ih, :],
                            start=(ih == 0),
                            stop=(ih == 1),
                        )
                    return mm

                def cp():
                    nc.scalar.copy(
                        s_sb[0:1, k * (SBLK * BC):(k + 1) * (SBLK * BC)],
                        sp[:])

                for half in range(2):
                    for gi in range(NG):
                        for ih in range(2):
                            fillers.append(mk_mm(half, gi, ih))
                fillers.append(cp)

            def filler_tick(n=None):
                if n is None:
                    n = 2 if len(fillers) > 20 else 1
                for _ in range(n):
                    if fillers:
                        fillers.pop(0)()

            # ------------- prologue: chunks 0..LEAD-1 densely -------------
            for c in range(LEAD):
                push_proj_chunk(c)
            while fillers:
                filler_tick(4)

            # ---------------- scan ----------------
            for gi in range(NG):
                for ih in range(2):
                    nc.vector.tensor_scalar_mul(
                        states[:, 0, gi, ih, :],
                        in0=gtiles[0][:, 0, gi, ih, :],
                        scalar1=expStart_sb[:, ih:ih + 1],
                    )

            for t in range(1, S):
                if t % SBLK == 1:
                    blk = t // SBLK
                    if blk + LEAD < NCHUNK:
                        push_proj_chunk(blk + LEAD)
                    if blk >= 1:
                        push_sblock(blk - 1)

                # filler BEFORE the scan groups: keeps [F][A-mms][B-mms]
                # order so B's matmuls abut A's and the two DVE multiplies
                # stay back-to-back
                filler_tick()
                gt = gtiles[t // SBLK]
                ti = t % SBLK
                for gi in range(NG):
                    ps = scan_ps.tile([128, 2, GB], f32, name=f"ps{gi}",
                                      tag="ps")
                    for jh in range(2):
                        for ih in range(2):
                            nc.tensor.matmul(
                                ps[:, jh, :],
                                lhsT=expT_sb[:, ih * 2 + jh, :],
                                rhs=states[:, t - 1, gi, ih, :],
                                start=(ih == 0),
                                stop=(ih == 1),
                            )
                    # flat [128, 32] multiply: ps and the states/G slices
                    # are contiguous 32-column runs
                    nc.vector.tensor_tensor(
                        out=states[:, t, gi, :, :],
                        in0=ps[:],
                        in1=gt[:, ti, gi, :, :],
                        op=mybir.AluOpType.mult,
                    )

            push_sblock(NCHUNK - 1)
            while fillers:
                filler_tick(4)

            nc.sync.dma_start(out=s_out[:], in_=s_sb[:])

    nc.compile()
    return nc


def _host_consts(d):
    W_ = np.asarray(d["W"], dtype=np.float32)
    b_ = np.asarray(d["b"], dtype=np.float64)
    T_ = np.asarray(d["transition"], dtype=np.float64)
    start_ = np.asarray(d["start_transition"], dtype=np.float64)
    end_ = np.asarray(d["end_transition"], dtype=np.float64)
    Wb = np.ascontiguousarray(
        W_.reshape(4, 128, 2, 128).transpose(1, 0, 2, 3).reshape(128, 8, 128)
    ).astype(bf16)
    expTb = np.ascontiguousarray(
        np.exp(T_).reshape(2, 128, 2, 128).transpose(1, 0, 2, 3).reshape(128, 4, 128)
    ).astype(bf16)
    biasT = np.ascontiguousarray(
        (b_ - KAPPA).reshape(2, 128).T).astype(np.float32)
    expStartT = np.ascontiguousarray(
        np.exp(start_).reshape(2, 128).T).astype(np.float32)
    expEndT = np.ascontiguousarray(
        np.exp(end_).reshape(2, 128).T).astype(bf16)
    return Wb, expTb, biasT, expStartT, expEndT


def _prep_core_inputs(core, enc_bf, Wb, expTb, biasT, expStartT, expEndT):
    # encT layout [h%128, chunk, h//128, row-in-chunk]; rows are t*BC + b
    b0 = core * BC
    e = enc_bf[:, b0:b0 + BC, :].transpose(2, 0, 1).reshape(4, 128, NCHUNK, CHUNK)
    e = np.ascontiguousarray(e.transpose(1, 2, 0, 3))
    return {
        "encT": e, "wblk": Wb, "expTblk": expTb, "biasT": biasT,
        "expStartT": expStartT, "expEndT": expEndT,
    }


def kernel(enc_outs, W, b, transition, start_transition, end_transition,
           targets, lengths):
    global _nc_cache
    if _nc_cache is None:
        _nc_cache = _build()
    nc = _nc_cache

    enc = np.asarray(enc_outs, dtype=np.float32)
    W_ = np.asarray(W, dtype=np.float32)
    b_ = np.asarray(b, dtype=np.float64)
    T_ = np.asarray(transition, dtype=np.float64)
    start_ = np.asarray(start_transition, dtype=np.float64)
    end_ = np.asarray(end_transition, dtype=np.float64)
    tgt = np.asarray(targets).astype(np.int64)
    lens = np.asarray(lengths).astype(np.int64)

    Wb, expTb, biasT, expStartT, expEndT = _host_consts({
        "W": W, "b": b, "transition": transition,
        "start_transition": start_transition, "end_transition": end_transition,
    })
    enc_bf = enc.astype(bf16)
    in_maps = [
        _prep_core_inputs(c, enc_bf, Wb, expTb, biasT, expStartT, expEndT)
        for c in range(NCORES)
    ]
    res = run_bass_kernel_spmd(nc, in_maps, list(range(NCORES))).results

    # ---------------- host epilogue (small inputs only) ----------------
    tmask = (np.arange(S)[:, None] < lens[None, :])
    trans_sum = (T_[tgt[:-1], tgt[1:]] * tmask[1:]).sum(axis=0)
    last_tgt = tgt[lens - 1, np.arange(B)]
    hostscore = start_[tgt[0]] + trans_sum + end_[last_tgt]

    # gold-path raw emission scores: R[t, b, tgt] = enc[t, b] . W[:, tgt] + b
    # (16K dot products per core; 0.1% of the device FLOPs)
    Wg = W_.T[tgt.reshape(-1)]                        # (S*B, H)
    emis_all = (np.einsum("rh,rh->r", enc.reshape(S * B, H), Wg,
                          optimize=True).reshape(S, B)
                + b_[tgt])
    emis = ((emis_all - KAPPA) * tmask).sum(axis=0)

    loss_b = np.zeros(B, dtype=np.float64)
    for c in range(NCORES):
        b0 = c * BC
        # S col layout: (blk, half, g, ti, b') with t = blk*16+half*8+ti,
        # b = g*16 + b'
        s_dec = np.asarray(res[c]["s_out"], dtype=np.float64).reshape(
            S // SBLK, 2, NG, 8, GB)
        bl = lens[b0:b0 + BC] - 1
        blocal = np.arange(BC)
        s_end = s_dec[bl // SBLK, (bl % SBLK) // 8, blocal // GB,
                      bl % 8, blocal % GB]
        loss_b[b0:b0 + BC] = np.log(s_end) - emis[b0:b0 + BC] \
            - hostscore[b0:b0 + BC]

    return np.float32(loss_b.mean())
